# revision 46
# baseline (speedup 1.0000x reference)
"""TransformerXL relative attention on 8 TRN2 NeuronCores, data-parallel over batch.

Problem shapes (hardcoded): B=8, Q=512, M=512, R=1024, HIDDEN=1024, HEADS=16, SPH=64.
Each core computes one batch element end to end; no collectives.

Layout strategy: host passes transposed activations (refT/queryT/posT, [D, *]) so
every matmul has its contraction dim on partitions. rel_shift is exact via a padded
DRAM buffer: writing positions rows into [Q, R+1] (pad col 0) makes the shifted
tensor a contiguous read at element offset Q. The token mask is folded into the
padded buffer on the host (inverse-shifted), so masking costs nothing on device.
Softmax runs without max-subtraction (scores are O(+-30), exp is safe in f32).

Schedule: software-pipelined at head-pair granularity, two generators deep.
pass1a(p) (weight loads + kc/kp/q projections) drains one full window before
pass1b(p) (position scores -> qt-granular pad writes -> half-split transposed
reads -> V projection), which drains at the top of pass2(p-1)'s window so the
DRAM rel-shift round trip lands ~65% into the window. pass2 consumes
(u0 rb0-3, u1 rb0-3, u0 rb4-7, u1 rb4-7) to match the read arrival order, with
the shifted positions accumulated into the content PSUM via identity matmuls on
the otherwise-bottleneck-free PE queue, and fully-masked 128-column blocks of
content/ident/attnV/exp skipped (attnV uses per-column-block stop flags).
Stage C pre-accumulates pairs 0..6 of five output groups before pair 7's
normalize completes. Latency-critical DMAs ride the sync queue (ACT's exec
queue depth is 0, so scalar-queue DMAs stall behind every ACT op).
"""
import numpy as np
import ml_dtypes

HIDDEN = 1024
HEADS = 16
SPH = 64
B, Q, M = 8, 512, 512
R = Q + M
NEG_INF = -1e9
P = 128
NPAIR = 8   # head pairs
NQT = Q // P
NCH = HIDDEN // P
VW = 65  # 64 v columns + 1 ones column per head (softmax denominator)

_CACHE = {}


def _build_nc(n_iter=1):
    import concourse.bass as bass  # noqa: F401
    from concourse import bacc
    import concourse.tile as tile
    import concourse.mybir as mybir

    f32 = mybir.dt.float32
    bf16 = mybir.dt.bfloat16

    nc = bacc.Bacc("TRN2", target_bir_lowering=False, debug=False)

    refT_e = nc.declare_dram_parameter("refT", [HIDDEN, R], bf16, isOutput=False)
    queryT_e = nc.declare_dram_parameter("queryT", [HIDDEN, Q], bf16, isOutput=False)
    posT_e = nc.declare_dram_parameter("posT", [HIDDEN, R], bf16, isOutput=False)
    wq_e = nc.declare_dram_parameter("wq", [HIDDEN, HIDDEN], bf16, isOutput=False)
    wkc_e = nc.declare_dram_parameter("wkc", [HIDDEN, HIDDEN], bf16, isOutput=False)
    wkp_e = nc.declare_dram_parameter("wkp", [HIDDEN, HIDDEN], bf16, isOutput=False)
    wv_e = nc.declare_dram_parameter("wv", [HIDDEN, HIDDEN], bf16, isOutput=False)
    wo_e = nc.declare_dram_parameter("wo", [HIDDEN, HIDDEN], bf16, isOutput=False)
    cbp_e = nc.declare_dram_parameter("cbp", [P, NPAIR], f32, isOutput=False)
    pbp_e = nc.declare_dram_parameter("pbp", [P, NPAIR], f32, isOutput=False)
    mshift_e = nc.declare_dram_parameter("mshift", [Q, R], bf16, isOutput=False)
    mcol_e = nc.declare_dram_parameter("mcol", [Q, 1], bf16, isOutput=False)
    out_e = nc.declare_dram_parameter("out", [Q, HIDDEN], f32, isOutput=True)

    with tile.TileContext(nc) as tc:
        from contextlib import ExitStack
        ctx = ExitStack()
        dram = ctx.enter_context(tc.tile_pool(name="dram", bufs=1, space="DRAM"))
        # per-head padded DRAM buffers for the rel_shift round trip
        pads = [dram.tile([Q * (R + 1)], bf16, tag=f"pad{h}", name=f"pad{h}")
                for h in range(HEADS)]
        pad_rows = [t[:].rearrange("(q c) -> q c", c=R + 1) for t in pads]
        shift_views = [t[Q:Q + Q * R].rearrange("(q c) -> q c", c=R) for t in pads]
        const = ctx.enter_context(tc.tile_pool(name="const", bufs=1))
        resid = ctx.enter_context(tc.tile_pool(name="resid", bufs=1))
        wstream = ctx.enter_context(tc.tile_pool(name="wstream", bufs=2))
        psum = ctx.enter_context(tc.tile_pool(name="psum", bufs=1, space="PSUM"))
        work = ctx.enter_context(tc.tile_pool(name="work", bufs=2))
        small = ctx.enter_context(tc.tile_pool(name="small", bufs=2))

        import numpy as _np
        import ml_dtypes as _mld
        ident_d = nc.inline_tensor(_np.eye(P, dtype=_mld.bfloat16), name="ident_d")
        ident = const.tile([P, P], bf16, tag="ident", name="ident")

        state = {}
        for _it in range(n_iter):
            _build_body(nc, tc, mybir, ctx, const, resid, wstream, psum, work,
                        small, dram, pads, pad_rows, shift_views, state,
                        (cbp_e, pbp_e, mshift_e, posT_e, queryT_e,
                         mcol_e, refT_e, ident_d),
                        wq_e, wkc_e, wkp_e, wv_e, wo_e, out_e, ident)
        ctx.close()

    nc.compile()
    return nc


def _build_body(nc, tc, mybir, ctx, const, resid, wstream, psum, work, small,
                dram, pads, pad_rows, shift_views, state, deferred,
                wq_e, wkc_e, wkp_e, wv_e, wo_e, out_e, ident):
    f32 = mybir.dt.float32
    bf16 = mybir.dt.bfloat16
    EXP = mybir.ActivationFunctionType.Exp
    IDENT = mybir.ActivationFunctionType.Identity

    # ---- pair weights: per-name emission so pair-0's can interleave with
    # the resident loads ----
    wpair = {}

    def _emit_w(p, name, w_e):
        tt = wstream.tile([P, HIDDEN], bf16, tag=f"wp_{name}", bufs=3,
                          name=f"wp_{name}")
        nc.sync.dma_start(tt[:], w_e[p * P:(p + 1) * P, :])
        wpair.setdefault(p, {})[name] = tt

    def _load_pair_w(p):
        for name, w_e in (("kc", wkc_e), ("kp", wkp_e), ("q", wq_e)):
            _emit_w(p, name, w_e)

    # ---- one-time resident loads, interleaved with pair-0 weights so each
    # projection's inputs land as early as possible ----
    if not state:
        cbp_e, pbp_e, mshift_e, posT_e, queryT_e, mcol_e, refT_e, ident_d = \
            deferred
        _emit_w(0, "kc", wkc_e)
        refT_sb = resid.tile([P, NCH * R], bf16, tag="refT", name="refT")
        for h in range(2):
            nc.sync.dma_start(
                refT_sb[:, h * 4 * R:(h + 1) * 4 * R].rearrange(
                    "p (c r) -> p c r", r=R),
                refT_e[h * 512:(h + 1) * 512, :].rearrange(
                    "(c p) r -> p c r", p=P))
        nc.sync.dma_start(ident[:], ident_d[:, :])
        posT_sb = resid.tile([P, NCH * R], bf16, tag="posT", name="posT")
        nc.sync.dma_start(
            posT_sb[:, 0:4 * R].rearrange("p (c r) -> p c r", r=R),
            posT_e[0:512, :].rearrange("(c p) r -> p c r", p=P))
        _emit_w(0, "kp", wkp_e)
        nc.sync.dma_start(
            posT_sb[:, 4 * R:8 * R].rearrange("p (c r) -> p c r", r=R),
            posT_e[512:1024, :].rearrange("(c p) r -> p c r", p=P))
        queryT_sb = resid.tile([P, NCH * Q], bf16, tag="queryT", name="queryT")
        nc.sync.dma_start(
            queryT_sb[:].rearrange("p (c q) -> p c q", q=Q),
            queryT_e[:, :].rearrange("(c p) q -> p c q", p=P))
        _emit_w(0, "q", wq_e)
        mshift_sb = resid.tile([P, NQT * R], bf16, tag="mshift", name="mshift")
        nc.sync.dma_start(
            mshift_sb[:].rearrange("p (t r) -> p t r", r=R),
            mshift_e[:, :].rearrange("(t p) r -> p t r", p=P))
        cbp = const.tile([P, NPAIR], f32, tag="cbp", name="cbp")
        nc.sync.dma_start(cbp[:], cbp_e[:, :])
        pbp = const.tile([P, NPAIR], f32, tag="pbp", name="pbp")
        nc.sync.dma_start(pbp[:], pbp_e[:, :])
        with nc.allow_non_contiguous_dma(reason="one-time pad columns"):
            for hh in range(HEADS):
                nc.gpsimd.dma_start(pad_rows[hh][:, 0:1], mcol_e[:, :])
        state.update(cbp=cbp, pbp=pbp, mshift_sb=mshift_sb, refT_sb=refT_sb,
                     posT_sb=posT_sb, queryT_sb=queryT_sb)
    if 0 not in wpair:
        _load_pair_w(0)
    cbp = state["cbp"]; pbp = state["pbp"]
    mshift_sb = state["mshift_sb"]
    posT_sb = state["posT_sb"]; queryT_sb = state["queryT_sb"]
    refT_sb = state["refT_sb"]

    def refT(c):
        return refT_sb[:, c * R:(c + 1) * R]

    def posT(c):
        return posT_sb[:, c * R:(c + 1) * R]

    def queryT(c):
        return queryT_sb[:, c * Q:(c + 1) * Q]

    # ---- per-iteration streamed weights; halves so V(pairs 0-3) can start
    # as soon as the first 512 columns of each chunk land ----
    wv_sb = wstream.tile([P, NCH * HIDDEN], bf16, tag="wv", bufs=1, name="wv")
    for vg in range(2):
        nc.sync.dma_start(
            wv_sb[:].rearrange("p (c d) -> p c d", d=HIDDEN)[
                :, :, vg * 512:(vg + 1) * 512],
            wv_e[:, vg * 512:(vg + 1) * 512].rearrange(
                "(c p) d -> p c d", p=P))

    # ---- v_sb: [P, 16*65], col 65h+64 = 1 (softmax denominator ones) ----
    v_sb = []
    for rt in range(NCH):
        t = resid.tile([P, HEADS * VW], bf16, tag=f"v{rt}", name=f"v{rt}")
        nc.vector.memset(
            t[:].rearrange("p (h w) -> p h w", w=VW)[:, :, 64:65], 1.0)
        v_sb.append(t)

    oT_sb = []
    for p in range(NPAIR):
        oT_sb.append(resid.tile([P, Q], bf16, tag=f"oT{p}", name=f"oT{p}"))

    # ---------------------------------------------------------------
    # pass1 generator for pair p: projections, position scores, the pad
    # round trip, and this pair's V projection. Yields between PE-op
    # groups (~0.5-1us each) so pass2 of the previous pair can interleave
    # them into its content->attnV gaps.
    # ---------------------------------------------------------------
    def pass1a(p):
        # projections for pair p: weight loads, kc, kp, q. Drained one full
        # window ahead of pass1b(p) so the position scores can start at the
        # very top of the window that hides pair p's pad round trip.
        if p not in wpair:
            _load_pair_w(p)
        w = wpair[p]
        kc_sb = work.tile([P, R], bf16, tag="kc_sb", bufs=3)
        kp_sb = work.tile([P, R], bf16, tag="kp_sb", bufs=3)
        for rhalf in range(2):
            ps = psum.tile([P, 512], f32, tag="pps", bufs=3)
            for c in range(NCH):
                nc.tensor.matmul(ps[:], w["kc"][:, c * P:(c + 1) * P],
                                 refT(c)[:, rhalf * 512:(rhalf + 1) * 512],
                                 start=(c == 0), stop=(c == NCH - 1))
                if c == 3:
                    yield
            nc.vector.tensor_copy(kc_sb[:, rhalf * 512:(rhalf + 1) * 512],
                                  ps[:])
            yield
        for rhalf in range(2):
            ps = psum.tile([P, 512], f32, tag="pps", bufs=3)
            for c in range(NCH):
                nc.tensor.matmul(ps[:], w["kp"][:, c * P:(c + 1) * P],
                                 posT(c)[:, rhalf * 512:(rhalf + 1) * 512],
                                 start=(c == 0), stop=(c == NCH - 1))
                if c == 3:
                    yield
            nc.scalar.activation(kp_sb[:, rhalf * 512:(rhalf + 1) * 512],
                                 ps[:], IDENT, bias=0.0, scale=1.0)
            yield
        qc_sb = work.tile([P, Q], bf16, tag="qc_sb", bufs=3)
        qp_sb = work.tile([P, Q], bf16, tag="qp_sb", bufs=3)
        ps = psum.tile([P, 512], f32, tag="pps", bufs=3)
        for c in range(NCH):
            nc.tensor.matmul(ps[:], w["q"][:, c * P:(c + 1) * P],
                             queryT(c)[:], start=(c == 0), stop=(c == NCH - 1))
            if c == 3:
                yield
        nc.scalar.add(qc_sb[:], ps[:], cbp[:, p:p + 1])
        nc.scalar.add(qp_sb[:], ps[:], pbp[:, p:p + 1])
        state[f"proj{p}"] = (kc_sb, kp_sb, qc_sb, qp_sb)
        yield

    def pass1b(p, vgroup=None):
        kc_sb, kp_sb, qc_sb, qp_sb = state[f"proj{p}"]
        # position scores, pre-shift [q, j] layout, into merged pad tiles
        padm = {}
        for u in range(2):
            padm[u] = work.tile([P, NQT * R], bf16, tag="padm", bufs=2,
                                name=f"padm{u}")
        shAs = [None, None]
        for qt in range(NQT):
            for u in range(2):
                for jh in range(2):
                    pps = psum.tile([P, 512], f32, tag="pps", bufs=3)
                    if jh == 0 and qt < 2:
                        # j-blocks [128, 384-qt*128) are fully garbage AND
                        # only feed skipped post-shift tiles: don't compute
                        # them (block 0 stays - it carries the mask cells)
                        segs = [(0, 128), (384 - qt * P, 512)]
                    else:
                        segs = [(0, 512)]
                    for s0, s1 in segs:
                        nc.tensor.matmul(
                            pps[:, s0:s1],
                            qp_sb[u * 64:u * 64 + 64, qt * P:(qt + 1) * P],
                            kp_sb[u * 64:u * 64 + 64,
                                  jh * 512 + s0:jh * 512 + s1],
                            start=True, stop=True, skip_group_check=True)
                    for s0, s1 in segs:
                        dst = padm[u][:, qt * R + jh * 512 + s0:
                                      qt * R + jh * 512 + s1]
                        if jh == 0:
                            # mask cells only exist at j < 512 (j < Q-1-q)
                            nc.vector.tensor_add(
                                dst, pps[:, s0:s1],
                                mshift_sb[:, qt * R + s0:qt * R + s1])
                        else:
                            nc.scalar.activation(dst, pps[:, s0:s1], IDENT,
                                                 bias=0.0, scale=1.0)
                # qt-granular pad write on the sync queue (Activation's
                # exec-queue depth is 0, so scalar-queue DMAs would stall
                # behind every ACT engine op)
                nc.sync.dma_start(
                    pad_rows[2 * p + u][qt * P:(qt + 1) * P, 1:],
                    padm[u][:, qt * R:(qt + 1) * R])
            yield
        # transposed reads, half-granular, ordered u0h1,u1h1,u0h2,u1h2 to
        # match pass2's consumption order
        for u in range(2):
            shAs[u] = work.tile([P, NCH * 512], bf16, tag="shA", bufs=3,
                                name=f"shA{u}")
        for half in range(2):
            for u in range(2):
                nc.sync.dma_start(
                    shAs[u][:, half * 2048:(half + 1) * 2048].rearrange(
                        "pp (b q) -> pp b q", q=512),
                    shift_views[2 * p + u][:, half * 512:(half + 1) * 512],
                    transpose=True)
        yield

        # V projection for 4 pairs at a time (vgroup 0 -> pairs 0-3,
        # vgroup 1 -> pairs 4-7), full N=512 matmuls.
        if vgroup is not None:
            for rt in range(NCH):
                vps = psum.tile([P, 512], f32, tag="pps", bufs=3)
                for c in range(NCH):
                    nc.tensor.matmul(
                        vps[:],
                        refT(c)[:, rt * P:(rt + 1) * P],
                        wv_sb[:, c * HIDDEN + vgroup * 512:
                              c * HIDDEN + (vgroup + 1) * 512],
                        start=(c == 0), stop=(c == NCH - 1))
                    if c == 3:
                        yield
                dst = v_sb[rt][:, vgroup * 8 * VW:(vgroup + 1) * 8 * VW]
                dst = dst.rearrange("pp (h w) -> pp h w", w=VW)[:, :, 0:64]
                nc.scalar.activation(
                    dst, vps[:].rearrange("pp (h w) -> pp h w", w=64),
                    IDENT, bias=0.0, scale=1.0)
                yield

        state[f"shAs{p}"] = shAs
        state[f"qckc{p}"] = (qc_sb, kc_sb)
        state.pop(f"proj{p}", None)

    def _chain(gens):
        for g in gens:
            for x in g:
                yield x

    def drain(g, n=1):
        if g is None:
            return
        for _ in range(n):
            try:
                next(g)
            except StopIteration:
                break

    def drain_all(g):
        if g is None:
            return
        for _ in g:
            pass

    # stage-C prestart: during pass2(7) the pass1 pipeline is empty, so use
    # the idle drain slots to accumulate out-projection contributions from
    # pairs 0..6 (pair 7's lands after its normalize)
    def stagec_pre():
        for qt in range(2):
            for dhalf in range(2):
                ps = psum.tile([P, 512], f32, tag="pps", bufs=3)
                for c in range(NCH - 1):
                    nc.tensor.matmul(
                        ps[:], oT_sb[c][:, qt * P:(qt + 1) * P],
                        wo_sb[:, c * HIDDEN + dhalf * 512:
                              c * HIDDEN + (dhalf + 1) * 512],
                        start=(c == 0), stop=False)
                    if c in (2, 5):
                        yield
                state[f"scpre{qt}{dhalf}"] = ps
                yield

    # ---- prologue: projections+positions+V for pair 0, projections for 1 ----
    drain_all(pass1a(0))
    drain_all(pass1b(0, vgroup=0))
    drain_all(pass1a(1))

    wo_sb = None

    for p in range(NPAIR):
        shAs = state.pop(f"shAs{p}")
        qc_sb, kc_sb = state.pop(f"qckc{p}")
        gens = []
        if p + 1 < NPAIR:
            gens.append(pass1b(p + 1, vgroup=1 if p == 0 else None))
        if p + 2 < NPAIR:
            gens.append(pass1a(p + 2))
        if p + 1 >= NPAIR:
            gens.append(stagec_pre())
        gnext = _chain(gens)
        # front-load: positions(p+1) execute at the top of this window so
        # the pad round trip drains well before the next window needs it
        drain(gnext, 6)

        h0 = 2 * p
        opsTs = [psum.tile([VW, 512], f32, tag="opsT", bufs=2,
                           name=f"opsT{u}") for u in range(2)]
        eTs = {}

        def content(rb, u):
            # columns q < (rb-4)*128 are fully masked (r > M+q for the whole
            # 128-row r-block): skip computing them entirely
            c0 = max(0, rb - 4) * P
            cps = psum.tile([P, 512], f32, tag="cps", bufs=3,
                            name=f"cps{u}")
            nc.tensor.matmul(cps[:, c0:512],
                             kc_sb[u * 64:u * 64 + 64, rb * P:(rb + 1) * P],
                             qc_sb[u * 64:u * 64 + 64, c0:512],
                             start=True, stop=False, skip_group_check=True)
            # shifted positions ride in on the PE as an identity matmul:
            # cheaper than a DVE add and no cross-engine hop before exp
            nc.tensor.matmul(cps[:, c0:512], ident[:],
                             shAs[u][:, rb * 512 + c0:(rb + 1) * 512],
                             start=False, stop=True, skip_group_check=True)
            eT = work.tile([P, 512], bf16, tag="eT", bufs=3)
            nc.scalar.activation(eT[:, c0:512], cps[:, c0:512], EXP,
                                 bias=0.0, scale=1.0)
            eTs[(rb, u)] = eT

        def attnv(rb, u):
            eT = eTs.pop((rb, u))
            vslice = v_sb[rb][:, (h0 + u) * VW:(h0 + u + 1) * VW]
            if rb < 4:
                nc.tensor.matmul(opsTs[u][0:VW, :], vslice, eT[:],
                                 start=(rb == 0), stop=False,
                                 skip_group_check=True)
            else:
                # column block qb=rb-4 sees its last contribution here
                c0 = (rb - 4) * P
                nc.tensor.matmul(opsTs[u][0:VW, c0:c0 + P], vslice,
                                 eT[:, c0:c0 + P], start=False, stop=True,
                                 skip_group_check=True)
                if rb < NCH - 1:
                    nc.tensor.matmul(opsTs[u][0:VW, c0 + P:512], vslice,
                                     eT[:, c0 + P:512], start=False,
                                     stop=False, skip_group_check=True)

        order = ([(rb, 0) for rb in range(4)] + [(rb, 1) for rb in range(4)]
                 + [(rb, 0) for rb in range(4, NCH)]
                 + [(rb, 1) for rb in range(4, NCH)])
        content(*order[0])
        content(*order[1])
        for k, (rb, u) in enumerate(order):
            drain(gnext, 2)
            attnv(rb, u)
            if k + 2 < len(order):
                content(*order[k + 2])
        drain_all(gnext)

        # normalize: oT = opsT[0:64] * (1 / opsT[64])
        for u in range(2):
            rl = small.tile([1, 512], f32, tag="rl")
            nc.vector.reciprocal(rl[:], opsTs[u][64:65, :])
            rlb_sb = small.tile([64, 512], f32, tag="rlb_sb")
            nc.gpsimd.partition_broadcast(rlb_sb[:], rl[:])
            nc.vector.tensor_mul(oT_sb[p][u * 64:u * 64 + 64, :],
                                 opsTs[u][0:64, :], rlb_sb[:])

        if p == 4:
            # wo load dispatched late so it doesn't cut ahead of the
            # latency-critical pad round-trip DMAs; needed only at stage C
            wo_sb = wstream.tile([P, NCH * HIDDEN], bf16, tag="wo", bufs=1,
                                 name="wo")
            nc.sync.dma_start(
                wo_sb[:].rearrange("pp (c d) -> pp c d", d=HIDDEN),
                wo_e[:, :].rearrange("(c pp) d -> pp c d", pp=P))

    # ---- stage C: out = oT.T @ Wo ----
    # Groups (qt0,*) were pre-accumulated over pairs 0..6 during pass2(7).
    # Phase A: accumulate pairs 0..6 for three more groups now - this PE work
    # does not need oT[7], so it runs while pair 7's normalize drains.
    # Phase B: finish every held group with its pair-7 contribution.
    # Phase C: remaining groups in full.
    def _wo_mm(ps, qt, dhalf, c, start, stop):
        nc.tensor.matmul(
            ps[:], oT_sb[c][:, qt * P:(qt + 1) * P],
            wo_sb[:, c * HIDDEN + dhalf * 512:c * HIDDEN + (dhalf + 1) * 512],
            start=start, stop=stop)

    def _emit_out(ps, qt, dhalf):
        ot = work.tile([P, 512], f32, tag="ot", bufs=4)
        if dhalf == 0:
            nc.scalar.activation(ot[:], ps[:], IDENT, bias=0.0, scale=1.0)
        else:
            nc.vector.tensor_copy(ot[:], ps[:])
        nc.sync.dma_start(
            out_e[qt * P:(qt + 1) * P, dhalf * 512:(dhalf + 1) * 512], ot[:])

    held = {}
    for qt, dhalf in ((1, 0), (1, 1), (2, 0)):
        ps = psum.tile([P, 512], f32, tag="cps", bufs=3)
        for c in range(NCH - 1):
            _wo_mm(ps, qt, dhalf, c, start=(c == 0), stop=False)
        held[(qt, dhalf)] = ps
    for qt, dhalf in ((0, 0), (0, 1), (1, 0), (1, 1), (2, 0)):
        ps = state.pop(f"scpre{qt}{dhalf}", None) or held.pop((qt, dhalf))
        _wo_mm(ps, qt, dhalf, NCH - 1, start=False, stop=True)
        _emit_out(ps, qt, dhalf)
    for qt, dhalf in ((2, 1), (3, 0), (3, 1)):
        ps = psum.tile([P, 512], f32, tag="pps", bufs=3)
        for c in range(NCH):
            _wo_mm(ps, qt, dhalf, c, start=(c == 0), stop=(c == NCH - 1))
        _emit_out(ps, qt, dhalf)


def _get_nc(n_iter=1):
    key = f"nc{n_iter}"
    if key not in _CACHE:
        _CACHE[key] = _build_nc(n_iter)
    return _CACHE[key]


def prepare_in_maps(query_seqs, memory_seqs, positional_encoding, token_mask,
                    content_bias, position_bias, Wq, Wkc, Wkp, Wv, Wo):
    qs = np.asarray(query_seqs, np.float32)
    ms = np.asarray(memory_seqs, np.float32)
    pe = np.asarray(positional_encoding, np.float32)
    tm = np.asarray(token_mask, np.float32)
    scale = np.float32(1.0 / np.sqrt(SPH))

    ref = np.concatenate([ms, qs], axis=1)                      # [B, R, D]
    refT = np.ascontiguousarray(ref.transpose(0, 2, 1))          # [B, D, R]
    queryT = np.ascontiguousarray(qs.transpose(0, 2, 1))         # [B, D, Q]
    posT = np.ascontiguousarray(pe.T)                            # [D, R]
    posT_bf = posT.astype(ml_dtypes.bfloat16)

    bf = ml_dtypes.bfloat16

    def _pair_permute(w):
        # [D, H*S] -> rows p*128..(p+1)*128 = pair p's 128 columns, chunk-major:
        # w_pre[row, c*128+col] = w[c*128+row, p*128+col]
        return np.ascontiguousarray(
            w.reshape(NCH, P, NPAIR, P).transpose(2, 1, 0, 3).reshape(
                HIDDEN, HIDDEN))

    wq = _pair_permute(np.asarray(Wq, np.float32).reshape(HIDDEN, HIDDEN) * scale).astype(bf)
    wkc = _pair_permute(np.asarray(Wkc, np.float32).reshape(HIDDEN, HIDDEN)).astype(bf)
    wkp = _pair_permute(np.asarray(Wkp, np.float32).reshape(HIDDEN, HIDDEN)).astype(bf)
    wv = np.ascontiguousarray(np.asarray(Wv, np.float32).reshape(HIDDEN, HIDDEN)).astype(bf)
    wo = np.ascontiguousarray(np.asarray(Wo, np.float32).reshape(HIDDEN, HIDDEN)).astype(bf)

    cbs = (np.asarray(content_bias, np.float32) * scale).reshape(HIDDEN)
    pbs = (np.asarray(position_bias, np.float32) * scale).reshape(HIDDEN)
    cbp = np.ascontiguousarray(cbs.reshape(NPAIR, P).T)          # [128, 8]
    pbp = np.ascontiguousarray(pbs.reshape(NPAIR, P).T)

    # inverse-shifted mask: writing M' into the padded buffer makes the shifted
    # read come out as positions + mask_bias
    mb = (tm[0, 0] * np.float32(NEG_INF)).astype(np.float32)     # [Q, R]
    mp_flat = np.zeros(Q * (R + 1), np.float32)
    mp_flat[Q:] = mb.ravel()
    mp = mp_flat.reshape(Q, R + 1)
    mshift = mp[:, 1:].astype(ml_dtypes.bfloat16)
    mcol = np.ascontiguousarray(mp[:, 0:1]).astype(ml_dtypes.bfloat16)

    in_maps = []
    for b in range(B):
        in_maps.append({
            "refT": np.ascontiguousarray(refT[b]).astype(ml_dtypes.bfloat16),
            "queryT": np.ascontiguousarray(queryT[b]).astype(ml_dtypes.bfloat16),
            "posT": posT_bf,
            "wq": wq, "wkc": wkc, "wkp": wkp, "wv": wv, "wo": wo,
            "cbp": cbp, "pbp": pbp,
            "mshift": mshift, "mcol": mcol,
        })
    return in_maps


def kernel(query_seqs, memory_seqs, positional_encoding, token_mask,
           content_bias, position_bias, Wq, Wkc, Wkp, Wv, Wo):
    from concourse.bass_utils import run_bass_kernel_spmd
    in_maps = prepare_in_maps(query_seqs, memory_seqs, positional_encoding,
                              token_mask, content_bias, position_bias,
                              Wq, Wkc, Wkp, Wv, Wo)
    nc = _get_nc()
    res = run_bass_kernel_spmd(nc, in_maps, core_ids=list(range(B)))
    out = np.stack([np.asarray(res.results[i]["out"], np.float32)
                    for i in range(B)], axis=0)
    return out
